# revision 1
# baseline (speedup 1.0000x reference)
"""Deformable conv (DCNv2) Trainium2 Bass kernel.

Problem (hardcoded): x [8, 128, 64, 64] f32; offset/mask 3x3 convs (pad 1);
bilinear-gather im2col; GEMM with weights [256, 1152]; out [8, 256, 64, 64].

Sharding: data-parallel over batch N=8 across 8 NeuronCores (1 sample/core);
weights/conv params replicated.

Per-core pipeline (sample n):
  1. x -> SBUF; build zero-padded bf16 conv input xpad [128, 68*68] and a
     transposed padded image xt_pad [68*68, 128] bf16 in DRAM (pad ring = 2,
     so out-of-range bilinear corners read zeros -> no validity masks needed).
  2. PE: offset/mask conv as 9 shifted matmuls (27 output rows: dy 0-8,
     dx 9-17, mask 18-26).
  3. PE-transpose conv output to j-major [128 part = j%128, (q, t)] and do all
     coordinate math with full-width DVE ops: floor via round-trick, clamp to
     [-2, 64], fractional weights, mask*bilinear corner weights, int16 gather
     indices into xt_pad.
  4. Per (k, j-half): SWDGE dma_gather (transpose mode) fetches pixel-pairs
     [(y,x),(y,x+1)] as [128ch, 2, J] bf16 tiles; corner weights are
     partition-broadcast from DRAM via step-0 APs; DVE: 2 muls + 3 adds
     produce the masked bilinear im2col chunk col_k [128, J] bf16.
  5. PE GEMM accumulates over k into PSUM [2x 128, J]; bias add; f32 out.
"""

import os
import numpy as np
import ml_dtypes

import concourse.bass as bass
import concourse.mybir as mybir
import concourse.tile as tile
from concourse import bacc
from concourse.bass_utils import run_bass_kernel_spmd
from concourse.masks import make_identity

F32 = mybir.dt.float32
BF16 = mybir.dt.bfloat16
I16 = mybir.dt.int16

N, C, H, W = 8, 128, 64, 64
K, K2, P = 3, 9, 256
HW = H * W                  # 4096
PW = W + 4                  # 68  (pad ring of 2)
ROWS = PW * PW              # 4624
NT = HW // 128              # 32 j-tiles of 128
KT = K2 * NT                # 288
JH = HW // 2                # 2048 per j-half
MAGIC = 12582912.0          # 1.5 * 2**23: fp32 round-to-int magic

_CACHE = {}


def _build_nc():
    nc = bacc.Bacc("TRN2", target_bir_lowering=False, debug=False,
                   num_devices=N, num_swdge_queues=4)

    x_in = nc.dram_tensor("x", [C, HW], F32, kind="ExternalInput")
    lhsT_om = nc.dram_tensor("lhsT_om", [C, K2, 32], BF16, kind="ExternalInput")
    lhsT_gemm = nc.dram_tensor("lhsT_gemm", [C, K2, P], BF16, kind="ExternalInput")
    basey = nc.dram_tensor("basey", [128, KT], F32, kind="ExternalInput")
    basex = nc.dram_tensor("basex", [128, KT], F32, kind="ExternalInput")
    bias_col = nc.dram_tensor("bias_col", [128, 2], F32, kind="ExternalInput")
    y_out = nc.dram_tensor("y", [P, HW], F32, kind="ExternalOutput")

    with tile.TileContext(nc) as tc:
        # DRAM scratch (one spare row so the overlapping pixel-pair view stays
        # in bounds; row ROWS is never addressed by an index).
        with tc.tile_pool(name="dram", bufs=1, space="DRAM") as dram:
            xt_pad = dram.tile([ROWS + 1, C], BF16)
            wrows = dram.tile([K2, 4, HW], BF16)
            idxw_dram = dram.tile([16, 2 * K2 * 2 * 128], I16)
            _emit(tc, nc, x_in, lhsT_om, lhsT_gemm, basey, basex,
                  bias_col, y_out, xt_pad, wrows, idxw_dram)
    nc.compile()
    return nc


def _emit(tc, nc, x_in, lhsT_om, lhsT_gemm, basey, basex, bias_col,
          y_out, xt_pad, wrows, idxw_dram):
    TS = nc.vector.tensor_scalar
    TT_ADD = nc.vector.tensor_add
    TT_SUB = nc.vector.tensor_sub
    TT_MUL = nc.vector.tensor_mul
    Alu = mybir.AluOpType

    with tc.tile_pool(name="singles", bufs=1) as singles:
        # ---- persistent tiles ----
        om_sb = singles.tile([C, K2, 32], BF16, tag="om", name="om")
        gemm_sb = singles.tile([C, K2, P], BF16, tag="gemm_w", name="gemm_w")
        bias_sb = singles.tile([128, 2], F32, tag="bias", name="bias")
        idx_wr = singles.tile([128, 2, K2, 2 * 128], I16, tag="idx_wr", name="idx_wr")
        ident = singles.tile([128, 128], BF16, tag="ident", name="ident")
        identf = singles.tile([32, 32], F32, tag="identf", name="identf")

        nc.sync.dma_start(out=om_sb, in_=lhsT_om[:])
        nc.sync.dma_start(out=gemm_sb, in_=lhsT_gemm[:])
        nc.sync.dma_start(out=bias_sb, in_=bias_col[:])
        make_identity(nc, ident)
        make_identity(nc, identf)

        with tc.tile_pool(name="stage1", bufs=1) as st1, \
             tc.tile_pool(name="coord", bufs=1) as coord, \
             tc.tile_pool(name="ps_a", bufs=2, space="PSUM") as ps_a, \
             tc.tile_pool(name="trbuf", bufs=2) as trbuf:

            # ---- stage 1: load x, build xpad (SBUF) and xt_pad (DRAM) ----
            xpad = st1.tile([C, ROWS], BF16, tag="xpad", name="xpad")
            x_sb = st1.tile([C, HW], F32, tag="x", name="x")
            nc.sync.dma_start(out=x_sb, in_=x_in[:])

            nc.vector.memset(xpad, 0.0)
            xpad_int = bass.AP(tensor=xpad.tensor,
                               offset=xpad.offset + 2 * PW + 2,
                               ap=[xpad.ap[0], [PW, H], [1, W]])
            nc.scalar.copy(out=xpad_int,
                           in_=x_sb[:].rearrange("c (h w) -> c h w", h=H))

            xbf = st1.tile([C, HW], BF16, tag="xbf", name="xbf")
            nc.vector.tensor_copy(xbf, x_sb)

            # zero xt_pad ([ROWS+1, C] = 592000 elems) via two overlapping
            # flat DMAs from a dedicated zero tile
            zt = st1.tile([128, 2320], BF16, tag="zt", name="zt")
            nc.vector.memset(zt, 0.0)
            zsrc = bass.AP(tensor=zt.tensor, offset=zt.offset,
                           ap=[[zt.ap[0][0], 128], [1, 2320]])
            half = 128 * 2320
            nc.sync.dma_start(
                out=bass.AP(tensor=xt_pad.tensor, offset=0, ap=[[1, half]]),
                in_=zsrc)
            nc.sync.dma_start(
                out=bass.AP(tensor=xt_pad.tensor,
                            offset=(ROWS + 1) * C - half, ap=[[1, half]]),
                in_=zsrc)

            # transpose x (bf16) 128-col chunks -> xt_pad interior
            for t in range(NT):
                tr_ps = ps_a.tile([128, 128], BF16, tag="trx", name="trx")
                nc.tensor.transpose(tr_ps[:], xbf[:, t * 128:(t + 1) * 128],
                                    ident[:])
                tr_sb = trbuf.tile([128, 128], BF16, tag="trx_sb", name="trx_sb")
                nc.scalar.copy(out=tr_sb, in_=tr_ps)
                dst = bass.AP(tensor=xt_pad.tensor,
                              offset=((2 * t + 2) * PW + 2) * C,
                              ap=[[PW * C, 2], [C, W], [1, C]])
                src = bass.AP(tensor=tr_sb.tensor, offset=tr_sb.offset,
                              ap=[[tr_sb.ap[0][0], 128], [1, 128]])
                nc.sync.dma_start(out=dst, in_=src)

            if os.environ.get("KDBG") == "X":
                for h2 in range(2):
                    xt_sb = st1.tile([128, 2320], BF16, tag=f"xt{h2}", name=f"xt{h2}")
                    half = 128 * 2320
                    src = bass.AP(tensor=xt_pad.tensor,
                                  offset=0 if h2 == 0 else (ROWS + 1) * C - half,
                                  ap=[[1, half]])
                    sdst = bass.AP(tensor=xt_sb.tensor, offset=xt_sb.offset,
                                   ap=[[xt_sb.ap[0][0], 128], [1, 2320]])
                    nc.sync.dma_start(out=sdst, in_=src)
                    xt_f = st1.tile([128, 2320], F32, tag=f"xtf{h2}", name=f"xtf{h2}")
                    nc.vector.tensor_copy(xt_f, xt_sb)
                    ydst = bass.AP(tensor=y_out,
                                   offset=0 if h2 == 0 else (ROWS + 1) * C - half,
                                   ap=[[1, half]])
                    sfsrc = bass.AP(tensor=xt_f.tensor, offset=xt_f.offset,
                                    ap=[[xt_f.ap[0][0], 128], [1, 2320]])
                    nc.sync.dma_start(out=ydst, in_=sfsrc)
                return

            # ---- stage 2: offset/mask conv (27 out rows), 1024-col chunks ----
            co_sb = st1.tile([32, HW], F32, tag="co", name="co")
            for nt8 in range(8):
                co_ps = ps_a.tile([32, 512], F32, tag="conv", name="conv")
                for tap in range(K2):
                    dy, dx = tap // K, tap % K
                    rhs = bass.AP(
                        tensor=xpad.tensor,
                        offset=(xpad.offset + (1 + dy) * PW + (1 + dx)
                                + (nt8 * 8) * PW),
                        ap=[xpad.ap[0], [PW, 8], [1, W]],
                    )
                    nc.tensor.matmul(co_ps[:], om_sb[:, tap, :], rhs,
                                     start=(tap == 0), stop=(tap == K2 - 1))
                nc.scalar.copy(out=co_sb[:, nt8 * 512:(nt8 + 1) * 512],
                               in_=co_ps)

            # ---- stage 3: transpose conv out to j-major; coordinate math ----
            trj = coord.tile([128, 32, NT], F32, tag="trj", name="trj")   # [jp, q, t]
            for t in range(NT):
                tp = ps_a.tile([128, 32], F32, tag="trjp", name="trjp")
                nc.tensor.transpose(tp[:], co_sb[:, t * 128:(t + 1) * 128],
                                    identf[:])
                nc.vector.tensor_copy(trj[:, :, t], tp)

            dy_all = trj[:, 0:K2, :]
            dx_all = trj[:, K2:2 * K2, :]
            m_all = trj[:, 2 * K2:3 * K2, :]

            by = coord.tile([128, KT], F32, tag="by", name="by")
            bx = coord.tile([128, KT], F32, tag="bx", name="bx")
            nc.sync.dma_start(out=by, in_=basey[:])
            nc.sync.dma_start(out=bx, in_=basex[:])

            def f32t(tag):
                return coord.tile([128, KT], F32, tag=tag, name=tag)

            py = f32t("py"); TT_ADD(py, dy_all, by)
            px = f32t("px"); TT_ADD(px, dx_all, bx)
            ty = f32t("ty"); TS(out=ty, in0=py, scalar1=-0.5, scalar2=MAGIC,
                                op0=Alu.add, op1=Alu.add)
            y0 = f32t("y0"); TS(out=y0, in0=ty, scalar1=MAGIC, scalar2=None,
                                op0=Alu.subtract)
            tx = f32t("tx"); TS(out=tx, in0=px, scalar1=-0.5, scalar2=MAGIC,
                                op0=Alu.add, op1=Alu.add)
            x0 = f32t("x0"); TS(out=x0, in0=tx, scalar1=MAGIC, scalar2=None,
                                op0=Alu.subtract)
            ly = f32t("ly"); TT_SUB(ly, py, y0)
            lx = f32t("lx"); TT_SUB(lx, px, x0)
            y0c = f32t("y0c"); TS(out=y0c, in0=y0, scalar1=-2.0, scalar2=64.0,
                                  op0=Alu.max, op1=Alu.min)
            x0c = f32t("x0c"); TS(out=x0c, in0=x0, scalar1=-2.0, scalar2=64.0,
                                  op0=Alu.max, op1=Alu.min)

            # idx0 = (y0c+2)*68 + (x0c+2); idx1 = idx0 + 68
            ia = f32t("ia"); TS(out=ia, in0=y0c, scalar1=float(PW),
                                scalar2=float(2 * PW + 2),
                                op0=Alu.mult, op1=Alu.add)
            idx0f = f32t("idx0f"); TT_ADD(idx0f, ia, x0c)
            idx1f = f32t("idx1f"); TS(out=idx1f, in0=idx0f, scalar1=float(PW),
                                      scalar2=None, op0=Alu.add)
            idx_i16 = coord.tile([128, 2, KT], I16, tag="idx_i16", name="idx_i16")
            nc.vector.tensor_copy(idx_i16[:, 0, :], idx0f)
            nc.vector.tensor_copy(idx_i16[:, 1, :], idx1f)

            # mask * bilinear corner weights (mask = 2*sigmoid(conv))
            sig = f32t("sig")
            nc.scalar.activation(out=sig, in_=m_all,
                                 func=mybir.ActivationFunctionType.Sigmoid)
            m2 = f32t("m2"); TS(out=m2, in0=sig, scalar1=2.0, scalar2=None,
                                op0=Alu.mult)
            mly = f32t("mly"); TT_MUL(mly, m2, ly)
            muy = f32t("muy"); TT_SUB(muy, m2, mly)
            w11 = f32t("w11"); TT_MUL(w11, mly, lx)
            w10 = f32t("w10"); TT_SUB(w10, mly, w11)
            w01 = f32t("w01"); TT_MUL(w01, muy, lx)
            w00 = f32t("w00"); TT_SUB(w00, muy, w01)

            # weights -> bf16 -> DRAM wrows[k, r, p, t] (t contiguous)
            for r, wt in enumerate((w00, w01, w10, w11)):
                wb = coord.tile([128, KT], BF16, tag=f"wb{r}", name=f"wb{r}")
                nc.vector.tensor_copy(wb, wt)
                dst = bass.AP(tensor=wrows.tensor,
                              offset=wrows.offset + r * 128 * NT,
                              ap=[[NT, 128], [4 * 128 * NT, K2], [1, NT]])
                src = bass.AP(tensor=wb.tensor, offset=wb.offset,
                              ap=[[wb.ap[0][0], 128], [NT, K2], [1, NT]])
                nc.sync.dma_start(out=dst, in_=src)

            # wrap indices for dma_gather. Descriptor order is the permuted
            # sigma-order i = a*512 + t*16 + b <-> j = t*128 + 16a + b, so the
            # wrapped layout [i%16, i//16] = [b, a*32 + t] is built from
            # idx_i16 with contiguous 32-element t-runs (per a, per group g):
            # idx_wr[16g + b, pair, k, a*32 + t] = idx_i16[16a + b, pair, (k, t)]
            # build one wrapped group in DRAM (8 DMAs), then one broadcast
            # load replicates it across the 8 Q7 core groups
            FW = 2 * K2 * 2 * 128      # wrapped free size per partition row
            for a in range(8):
                dst = bass.AP(tensor=idxw_dram.tensor,
                              offset=idxw_dram.offset + a * NT,
                              ap=[[FW, 16], [2 * 128, 2 * K2], [1, NT]])
                sb = idx_i16[16 * a:16 * (a + 1), :, :]
                src = bass.AP(tensor=sb.tensor, offset=sb.offset,
                              ap=[sb.ap[0], [NT, 2 * K2], [1, NT]])
                nc.sync.dma_start(out=dst, in_=src)
            bsrc = bass.AP(tensor=idxw_dram.tensor, offset=idxw_dram.offset,
                           ap=[[0, 8], [FW, 16], [1, FW]])
            idst = bass.AP(tensor=idx_wr.tensor, offset=idx_wr.offset,
                           ap=[[idx_wr.ap[0][0], 128], [1, FW]])
            nc.sync.dma_start(out=idst, in_=bsrc)

            if os.environ.get("KDBG") == "A":
                dst = bass.AP(tensor=y_out, offset=0, ap=[[KT, 128], [1, KT]])
                nc.sync.dma_start(out=dst, in_=w00)
                idxf = coord.tile([128, 2, KT], F32, tag="idxf", name="idxf")
                nc.vector.tensor_copy(idxf, idx_i16)
                dst2 = bass.AP(tensor=y_out, offset=128 * KT,
                               ap=[[2 * KT, 128], [1, 2 * KT]])
                src2 = bass.AP(tensor=idxf.tensor, offset=idxf.offset,
                               ap=[[idxf.ap[0][0], 128], [1, 2 * KT]])
                nc.sync.dma_start(out=dst2, in_=src2)
                return

        # ---- stages 4+5: per j-half: gather, interpolate, GEMM ----
        gsrc = bass.AP(tensor=xt_pad.tensor, offset=xt_pad.offset,
                       ap=[[C, ROWS], [1, 2 * C]])

        ones_sb = singles.tile([1, 128], BF16, tag="ones", name="ones")
        nc.vector.memset(ones_sb, 1.0)

        with tc.tile_pool(name="gw", bufs=2) as gw, \
             tc.tile_pool(name="colp", bufs=1) as colp, \
             tc.tile_pool(name="outp", bufs=2) as outp, \
             tc.tile_pool(name="ps_b", bufs=1, space="PSUM") as ps_b:

            for jh in range(2):
                cols = []
                for k in range(K2):
                    # sigma-sliced weight rows for this (k, jh):
                    # wr_sb[0, r, ab, t] = w_r(k, j = 128t + 64jh + ab)
                    wr_sb = gw.tile([1, 4, 64, NT], BF16, tag="wr", name="wr",
                                    bufs=2)
                    wsrc = bass.AP(tensor=wrows.tensor,
                                   offset=(wrows.offset + k * 4 * 128 * NT
                                           + jh * 64 * NT),
                                   ap=[[128 * NT, 4], [NT, 64], [1, NT]])
                    nc.sync.dma_start(out=wr_sb, in_=wsrc)
                    g_t = []
                    w_t = []
                    for pair in range(2):
                        g = gw.tile([128, 2, JH], BF16, tag=f"g{pair}",
                                    name=f"g{pair}")
                        nc.gpsimd.dma_gather(
                            out_ap=g[:],
                            in_ap=gsrc,
                            idxs_ap=idx_wr[:, pair, k,
                                           jh * 128:(jh + 1) * 128],
                            num_idxs=JH,
                            num_idxs_reg=JH,
                            elem_size=2 * C,
                            elem_step=C,
                            transpose=True,
                            single_packet=False,
                            queue_num=0,
                        )
                        g_t.append(g)
                        # broadcast w rows (2*pair, 2*pair+1) across
                        # partitions in sigma-order: ones[1,128].T @
                        # wrow[1,512-chunk] -> PSUM, then ACT cast to bf16
                        wt = gw.tile([128, 2, JH], BF16, tag=f"w{pair}",
                                     name=f"w{pair}", bufs=2)
                        for s in range(2):
                            for h2 in range(2):
                                bc_ps = ps_b.tile([128, JH // 2], F32,
                                                  tag="bc", name="bc", bufs=2)
                                for nn in range(2):
                                    n2 = h2 * 2 + nn
                                    rhs = bass.AP(
                                        tensor=wr_sb.tensor,
                                        offset=(wr_sb.offset
                                                + (2 * pair + s) * 64 * NT
                                                + 16 * n2 * NT),
                                        ap=[[wr_sb.ap[0][0], 1],
                                            [1, NT], [NT, 16]])
                                    nc.tensor.matmul(
                                        bc_ps[:, nn * 512:(nn + 1) * 512],
                                        ones_sb[:], rhs,
                                        start=True, stop=True,
                                    )
                                nc.scalar.copy(
                                    out=wt[:, s, h2 * 1024:(h2 + 1) * 1024],
                                    in_=bc_ps)
                        w_t.append(wt)
                    # products in place over the gathered tiles
                    TT_MUL(g_t[0], g_t[0], w_t[0])
                    TT_MUL(g_t[1], g_t[1], w_t[1])
                    a_s = outp.tile([128, JH], BF16, tag="a", name="a")
                    col_k = colp.tile([128, JH], BF16, tag=f"col{k}",
                                      name=f"col{k}")
                    TT_ADD(a_s, g_t[0][:, 0, :], g_t[0][:, 1, :])
                    TT_ADD(col_k, g_t[1][:, 0, :], g_t[1][:, 1, :])
                    TT_ADD(col_k, col_k, a_s)
                    cols.append(col_k)
                for m in range(2):
                    ps_n = [ps_b.tile([128, 512], F32, tag=f"gemm{n2}",
                                      name=f"gemm{n2}") for n2 in range(4)]
                    for k in range(K2):
                        for n2 in range(4):
                            nc.tensor.matmul(
                                ps_n[n2][:],
                                gemm_sb[:, k, m * 128:(m + 1) * 128],
                                cols[k][:, n2 * 512:(n2 + 1) * 512],
                                start=(k == 0), stop=(k == K2 - 1),
                            )
                    # epilogue: bias add + sigma -> j-order unpermute via
                    # strided TS out APs, then one strided store to y
                    o_sb = outp.tile([128, JH], F32, tag="o", name="o")
                    for n2 in range(4):
                        o_ap = bass.AP(tensor=o_sb.tensor,
                                       offset=o_sb.offset + 16 * n2,
                                       ap=[o_sb.ap[0], [64, NT], [1, 16]])
                        TS(out=o_ap, in0=ps_n[n2],
                           scalar1=bias_sb[:, m:m + 1], scalar2=None,
                           op0=Alu.add)
                    dst = bass.AP(tensor=y_out,
                                  offset=m * 128 * HW + jh * 64,
                                  ap=[[HW, 128], [128, NT], [1, 64]])
                    nc.sync.dma_start(out=dst, in_=o_sb)


def _host_constants():
    if "consts" in _CACHE:
        return _CACHE["consts"]
    t_idx = np.arange(NT)
    p_idx = np.arange(128)
    j = t_idx[None, :] * 128 + p_idx[:, None]          # [128, 32]
    iy = j // W
    ix = j % W
    ky = np.repeat(np.arange(K), K)
    kx = np.tile(np.arange(K), K)
    basey = np.zeros((128, KT), dtype=np.float32)
    basex = np.zeros((128, KT), dtype=np.float32)
    for k in range(K2):
        basey[:, k * NT:(k + 1) * NT] = iy - 1 + ky[k]
        basex[:, k * NT:(k + 1) * NT] = ix - 1 + kx[k]
    _CACHE["consts"] = (basey, basex)
    return _CACHE["consts"]


def kernel(x, offset_w, offset_b, mask_w, mask_b, weights, bias):
    x = np.asarray(x, dtype=np.float32)
    offset_w = np.asarray(offset_w, dtype=np.float32)
    mask_w = np.asarray(mask_w, dtype=np.float32)
    weights = np.asarray(weights, dtype=np.float32)
    bias = np.asarray(bias, dtype=np.float32)
    offset_b = np.asarray(offset_b, dtype=np.float32)
    mask_b = np.asarray(mask_b, dtype=np.float32)
    assert np.all(offset_b == 0) and np.all(mask_b == 0), "zero conv bias assumed"

    if "nc" not in _CACHE:
        _CACHE["nc"] = _build_nc()
    nc = _CACHE["nc"]
    basey, basex = _host_constants()

    # offset/mask conv stationary operand [c, tap, q]: q 0-8 dy, 9-17 dx, 18-26 m
    lhsT_om = np.zeros((C, K2, 32), dtype=np.float32)
    ow = offset_w.reshape(K2, 2, C, K, K)
    for tap in range(K2):
        dy, dx = tap // K, tap % K
        lhsT_om[:, tap, 0:K2] = ow[:, 0, :, dy, dx].T
        lhsT_om[:, tap, K2:2 * K2] = ow[:, 1, :, dy, dx].T
        lhsT_om[:, tap, 2 * K2:3 * K2] = mask_w[:, :, dy, dx].T
    lhsT_om = lhsT_om.astype(ml_dtypes.bfloat16)

    # GEMM stationary operand: lhsT_gemm[k, c, p] = weights[p, c*9 + k]
    wr = weights.reshape(P, C, K2)
    lhsT_gemm = np.ascontiguousarray(wr.transpose(1, 2, 0)).astype(ml_dtypes.bfloat16)

    bias_col = np.ascontiguousarray(bias.reshape(2, 128).T).astype(np.float32)

    in_maps = []
    for n in range(N):
        in_maps.append({
            "x": np.ascontiguousarray(x[n].reshape(C, HW)),
            "lhsT_om": lhsT_om,
            "lhsT_gemm": lhsT_gemm,
            "basey": basey,
            "basex": basex,
            "bias_col": bias_col,
        })

    res = run_bass_kernel_spmd(nc, in_maps, core_ids=list(range(N)),
                               trace=bool(_CACHE.get("trace")),
                               trace_cores=_CACHE.get("trace_cores"))
    _CACHE["last_res"] = res
    out = np.stack([res.results[n]["y"].reshape(P, H, W) for n in range(N)])
    return out.astype(np.float32)

